# revision 1
# baseline (speedup 1.0000x reference)
"""Multi-head causal attention (B=2, S=2048, E=1024, H=16, D=64) on 8 TRN2 cores.

Sharding: core c -> batch b = c // 4, head group g = c % 4 (4 heads each).
Each core computes q/k/v projections + RoPE + causal attention + its rows of
the Wo projection for its (batch, head-group); the host sums the 4 row-parallel
Wo partials per batch (the unshard step of row-parallel output projection).

Device layout notes:
  - x is passed pre-transposed per batch: xT [E, S] so the PE can contract
    over E (partition dim) for the projections.
  - q/k are computed transposed (qT/kT [64, S]) with head-pair fused weights
    so one [128, 512] PSUM tile holds [q_x1; q_x2; k_x1; k_x2] rows, where
    x1/x2 are the RoPE even/odd pair halves (weight columns pre-permuted on
    host so rotate-half applies).
  - scores are computed transposed, sT [k, q] = kT.T @ qT; softmax runs over
    the partition dim via an appended ones-column in the AV matmul (Z row).
    No max-subtraction: scores ~ N(0,1), exp is safe in fp32.
  - AV computes attnT [d, q]; Wo projection contracts head-dim chunks of
    attnT against Wo rows (fp16), accumulating out [s, e] tiles in PSUM.
"""

import sys

if "/opt/trn_rl_repo" not in sys.path:
    sys.path.insert(0, "/opt/trn_rl_repo")

import numpy as np

import concourse.bass as bass
import concourse.tile as tile
from concourse import bacc, mybir
from concourse.bass_utils import run_bass_kernel_spmd

B, S, E, H, D = 2, 2048, 1024, 16, 64
HPC = 4  # heads per core
NCORES = 8
SB = 512  # q/s block width
NSB = S // SB  # 4
KT = 128  # k tile (partition chunk of the sequence)
NKT = S // KT  # 16
ECH = E // 128  # 8 contraction chunks for the projections

f32 = mybir.dt.float32
f16 = mybir.dt.float16
bf16 = mybir.dt.bfloat16

ROPE_BASE = 10000.0


def build_nc(unroll=1):
    nc = bacc.Bacc(
        "TRN2", target_bir_lowering=False, debug=False, enable_asserts=False
    )

    xT_d = nc.dram_tensor("xT", [E, S], f16, kind="ExternalInput")
    wqk_d = nc.dram_tensor("wqk", [E, HPC, 128], f16, kind="ExternalInput")
    wv_d = nc.dram_tensor("wv", [E, HPC * D], f16, kind="ExternalInput")
    wo_d = nc.dram_tensor("wo", [HPC * D, E], f16, kind="ExternalInput")
    cos_d = nc.dram_tensor("cos2", [128, S], f32, kind="ExternalInput")
    sin_d = nc.dram_tensor("sin2", [128, S], f32, kind="ExternalInput")
    mask_d = nc.dram_tensor("maskb", [128, 4, SB], f16, kind="ExternalInput")
    tri_d = nc.dram_tensor("tri", [128, 128], f16, kind="ExternalInput")
    out_d = nc.dram_tensor("out", [S, E], f16, kind="ExternalOutput")

    with tile.TileContext(nc) as tc:
        with (
            tc.tile_pool(name="const", bufs=1) as constp,
            tc.tile_pool(name="qk", bufs=1) as qkp,
            tc.tile_pool(name="vb", bufs=1) as vbp,
            tc.tile_pool(name="at", bufs=1) as atp,
            tc.tile_pool(name="st", bufs=12) as stp,
            tc.tile_pool(name="tmp", bufs=4) as tmpp,
            tc.tile_pool(name="mm", bufs=2, space="PSUM") as mmp,
            tc.tile_pool(name="wps", bufs=2, space="PSUM") as wpsp,
            tc.tile_pool(name="acc", bufs=1, space="PSUM") as accp,
        ):
            # ---- constant tiles (DMAs issued per s-block, in consumption
            # order, so the first projection matmuls start within a few us) --
            xT_ap = xT_d.ap().rearrange("(eo p) s -> eo p s", p=128)
            xT = [
                constp.tile([128, S], f16, tag=f"xT{e}", name=f"xT{e}")
                for e in range(ECH)
            ]
            wqk = constp.tile([128, ECH, HPC, 128], f16, tag="wqk")
            nc.sync.dma_start(
                out=wqk, in_=wqk_d.ap().rearrange("(eo p) h m -> p eo h m", p=128)
            )
            wv = constp.tile([128, ECH, HPC * D], f16, tag="wv")
            nc.sync.dma_start(
                out=wv, in_=wv_d.ap().rearrange("(eo p) m -> p eo m", p=128)
            )
            cos2 = constp.tile([128, S], f32, tag="cos2")
            sin2 = constp.tile([128, S], f32, tag="sin2")
            maskb = constp.tile([128, 4, SB], f16, tag="maskb")
            tri = constp.tile([128, 128], f16, tag="tri")
            wo = constp.tile([128, 2, E], f16, tag="wo")

            def emit_loads(sb):
                cs = slice(sb * SB, (sb + 1) * SB)
                for e in range(ECH):
                    nc.sync.dma_start(out=xT[e][:, cs], in_=xT_ap[e][:, cs])
                nc.sync.dma_start(out=cos2[:, cs], in_=cos_d.ap()[:, cs])
                nc.sync.dma_start(out=sin2[:, cs], in_=sin_d.ap()[:, cs])
                if sb == 0:
                    nc.sync.dma_start(out=maskb, in_=mask_d.ap())
                    nc.sync.dma_start(out=tri, in_=tri_d.ap())
                if sb == 1:
                    nc.sync.dma_start(
                        out=wo, in_=wo_d.ap().rearrange("(c p) e -> p c e", p=128)
                    )

            # qq[p] rows: qT of head 2p on partitions 0-63, head 2p+1 on 64-127
            # (kk[p] likewise) so each head's scores matmul operands share a
            # partition base. psum rows per head: [q_x1; q_x2; k_x1; k_x2].
            qq = [
                qkp.tile([128, S], f16, tag=f"qq{p}", name=f"qq{p}")
                for p in range(2)
            ]
            kk = [
                qkp.tile([128, S], f16, tag=f"kk{p}", name=f"kk{p}")
                for p in range(2)
            ]
            swap_src = [32, 0, 96, 64]

            def emit_qk_proj(sb):
                cs = slice(sb * SB, (sb + 1) * SB)
                for h in range(HPC):
                    p, half = h // 2, (h % 2) * 64
                    ps = mmp.tile([128, SB], f32, tag="mm", name="ps")
                    for e in range(ECH):
                        nc.tensor.matmul(
                            out=ps,
                            lhsT=wqk[:, e, h, :],
                            rhs=xT[e][:, cs],
                            start=(e == 0),
                            stop=(e == ECH - 1),
                        )
                    rs = tmpp.tile([128, SB], f32, tag="rs", name="rs")
                    nc.scalar.copy(out=rs, in_=ps)
                    t1 = tmpp.tile([128, SB], f32, tag="t1", name="t1")
                    t2 = tmpp.tile([128, SB], f32, tag="t2", name="t2")
                    nc.vector.tensor_mul(t1, rs, cos2[:, cs])
                    for g in range(4):
                        # sin2 rows are laid out so in0/in1 share a base
                        # partition (walrus SB+SB constraint)
                        srow = swap_src[g]
                        nc.vector.tensor_mul(
                            t2[g * 32 : (g + 1) * 32, :],
                            rs[srow : srow + 32, :],
                            sin2[srow : srow + 32, cs],
                        )
                    nc.vector.tensor_add(
                        qq[p][half : half + 64, cs], t1[0:64, :], t2[0:64, :]
                    )
                    nc.vector.tensor_add(
                        kk[p][half : half + 64, cs], t1[64:128, :], t2[64:128, :]
                    )

            # v_big free layout per k-chunk: 4 heads x [v_h (64) | one (1)]
            v_big = vbp.tile([128, NKT, HPC * 65], f16, tag="vbig")
            ones_cols = v_big.rearrange("p n (h m) -> p n h m", h=HPC)[
                :, :, :, 64:65
            ]
            nc.vector.memset(ones_cols, 1.0)

            def emit_v_proj(sb):
                for kc in range(4 * sb, 4 * sb + 4):
                    vps = mmp.tile([128, HPC * D], f32, tag="mm", name="vps")
                    for e in range(ECH):
                        nc.tensor.matmul(
                            out=vps,
                            lhsT=xT[e][:, kc * KT : (kc + 1) * KT],
                            rhs=wv[:, e, :],
                            start=(e == 0),
                            stop=(e == ECH - 1),
                        )
                    nc.vector.tensor_copy(
                        out=v_big.rearrange("p n (h m) -> p n h m", h=HPC)[
                            :, kc, :, 0:64
                        ],
                        in_=vps.rearrange("p (h m) -> p h m", h=HPC),
                    )

            # ---- phase C: attention per (q block, head pair) --------------------
            # attnT tiles: at8[c][qb] rows = hd chunk c (2 heads x 64), cols = q
            # Heads 2p / 2p+1 sit at partition bases 0 / 64 of qq[p]/kk[p], so
            # their K=64 scores matmuls land in disjoint PE row groups and run
            # concurrently (row tiling via auto tile_position).
            at8 = {}
            for c in range(2):
                for qb in range(NSB):
                    at8[(c, qb)] = atp.tile(
                        [128, SB], f16, tag=f"at{c}_{qb}", name=f"at{c}_{qb}"
                    )

            def emit_attn(qb):
                qs = slice(qb * SB, (qb + 1) * SB)
                n_k = 4 * (qb + 1)
                for p in range(2):
                    # one wide [128, 1024] PSUM pair-tile per head pair: both
                    # heads' scores live side by side so a single ACT exp
                    # covers them (halves exp instructions and sem hops)
                    av2 = accp.tile([128, 2 * SB], f32, tag="acc", name="av2")
                    # Software pipeline: emit the AV matmul for chunk kt only
                    # LAG steps after its scores matmul, so the PE (strict
                    # in-order queue) never head-of-line blocks on the ACT exp.
                    LAG = 2
                    sts_buf = {}
                    for step in range(n_k + LAG):
                        if step < n_k:
                            kt = step
                            j = kt - 4 * qb
                            kts = slice(kt * KT, (kt + 1) * KT)
                            ps2 = wpsp.tile(
                                [128, 2 * SB], f32, tag="wps", name="ps2"
                            )
                            for i in range(2):
                                half = i * 64
                                nc.tensor.matmul(
                                    out=ps2[:, i * SB : (i + 1) * SB],
                                    lhsT=kk[p][half : half + 64, kts],
                                    rhs=qq[p][half : half + 64, qs],
                                    start=True,
                                    stop=(j < 0),
                                )
                                if j >= 0:
                                    # causal mask: add -240*max(0, r+128j-c)
                                    # (tri.T @ maskb_j); exp(0.125*x) -> 0
                                    nc.tensor.matmul(
                                        out=ps2[:, i * SB : (i + 1) * SB],
                                        lhsT=tri,
                                        rhs=maskb[:, j, :],
                                        start=False,
                                        stop=True,
                                    )
                            st_t = stp.tile(
                                [128, 2 * SB], f16, tag="st", name="st_t"
                            )
                            nc.scalar.activation(
                                out=st_t,
                                in_=ps2,
                                func=mybir.ActivationFunctionType.Exp,
                                scale=0.125,
                            )
                            sts_buf[kt] = st_t
                        if step >= LAG:
                            kt = step - LAG
                            st_t = sts_buf.pop(kt)
                            for i in range(2):
                                h = 2 * p + i
                                nc.tensor.matmul(
                                    out=av2[0:65, i * SB : (i + 1) * SB],
                                    lhsT=v_big[:, kt, h * 65 : (h + 1) * 65],
                                    rhs=st_t[:, i * SB : (i + 1) * SB],
                                    start=(kt == 0),
                                    stop=(kt == n_k - 1),
                                )
                    # normalize: attnT = av[0:64] / Z  (Z = av row 64)
                    for i in range(2):
                        h = 2 * p + i
                        avi = av2[:, i * SB : (i + 1) * SB]
                        r = tmpp.tile([1, SB], f32, tag="r", name="r")
                        nc.vector.reciprocal(out=r, in_=avi[64:65, :])
                        zb = tmpp.tile([64, SB], f32, tag="zb", name="zb")
                        nc.gpsimd.partition_broadcast(zb, r)
                        c, half = h // 2, (h % 2) * 64
                        nc.vector.tensor_mul(
                            at8[(c, qb)][half : half + 64, :], avi[0:64, :], zb
                        )

            # ---- phase D: output projection (row-parallel partial) -------------
            def emit_out_proj(qb):
                for stl in range(4):
                    rows = qb * SB + stl * KT
                    for eb in range(2):
                        pw = mmp.tile([128, SB], f32, tag="mm", name="pw")
                        for c in range(2):
                            nc.tensor.matmul(
                                out=pw,
                                lhsT=at8[(c, qb)][:, stl * KT : (stl + 1) * KT],
                                rhs=wo[:, c, eb * SB : (eb + 1) * SB],
                                start=(c == 0),
                                stop=(c == 1),
                            )
                        ot = stp.tile([128, SB], f16, tag="ot", name="ot", bufs=3)
                        nc.vector.tensor_copy(out=ot, in_=pw)
                        nc.sync.dma_start(
                            out=out_d.ap()[rows : rows + KT, eb * SB : (eb + 1) * SB],
                            in_=ot,
                        )

            # ---- emission schedule: pipeline loads/proj with attention ----------
            # unroll > 1 repeats the whole kernel for overhead-free timing
            for _ in range(unroll):
                emit_loads(0)
                emit_qk_proj(0)
                emit_v_proj(0)
                emit_loads(1)
                emit_qk_proj(1)
                emit_v_proj(1)
                emit_attn(0)
                emit_loads(2)
                emit_qk_proj(2)
                emit_v_proj(2)
                emit_attn(1)
                emit_loads(3)
                emit_qk_proj(3)
                emit_v_proj(3)
                emit_out_proj(0)
                emit_attn(2)
                emit_out_proj(1)
                emit_attn(3)
                emit_out_proj(2)
                emit_out_proj(3)

    nc.compile()
    return nc


def build_in_maps(x, Wq, Wk, Wv, Wo):
    x = np.asarray(x, np.float32)
    Wq = np.asarray(Wq, np.float32)
    Wk = np.asarray(Wk, np.float32)
    Wv = np.asarray(Wv, np.float32)
    Wo = np.asarray(Wo, np.float32)

    # RoPE tables in rotate-half layout ([32] pair-frequencies, duplicated)
    inv = 1.0 / (ROPE_BASE ** (np.arange(0, D, 2, dtype=np.float64) / D))  # [32]
    ang = inv[:, None] * np.arange(S, dtype=np.float64)[None, :]  # [32, S]
    cos_t = np.cos(ang).astype(np.float32)
    sin_t = np.sin(ang).astype(np.float32)
    cos2 = np.concatenate([cos_t, cos_t, cos_t, cos_t], 0)  # [128, S]
    sin2 = np.concatenate([sin_t, -sin_t, sin_t, -sin_t], 0)  # [128, S] (rows at swap-source positions)

    # Causal mask matmul operands: accumulating tri.T @ maskb_j into the
    # scores psum adds -240*max(0, r + 128j - c), which the exp flushes to 0
    # exactly on the masked (k > q) region.
    tt = np.arange(128)[:, None]
    cc = np.arange(SB)[None, :]
    maskb = np.ascontiguousarray(
        np.stack([(cc < tt + j * KT) for j in range(4)], axis=1)
    ).astype(np.float16)  # [128, 4, SB]
    rr = np.arange(128)[None, :]
    tri = (-240.0 * (tt <= rr)).astype(np.float16)  # [t, r]

    # weight column permutation: even pair-elements then odd (rotate-half)
    perm = np.concatenate([np.arange(0, D, 2), np.arange(1, D, 2)])

    in_maps = []
    for core in range(NCORES):
        b, g = core // HPC, core % HPC
        wqk = np.empty((E, HPC, 128), np.float32)
        for i in range(HPC):
            h = g * HPC + i
            wqk[:, i, 0:64] = Wq[:, h * D : (h + 1) * D][:, perm]
            wqk[:, i, 64:128] = Wk[:, h * D : (h + 1) * D][:, perm]
        in_maps.append(
            {
                "xT": np.ascontiguousarray(x[b].T).astype(np.float16),
                "wqk": wqk.astype(np.float16),
                "wv": np.ascontiguousarray(
                    Wv[:, g * HPC * D : (g + 1) * HPC * D]
                ).astype(np.float16),
                "wo": np.ascontiguousarray(
                    Wo[g * HPC * D : (g + 1) * HPC * D, :]
                ).astype(np.float16),
                "cos2": cos2,
                "sin2": sin2,
                "maskb": maskb,
                "tri": tri,
            }
        )
    return in_maps


def gather_output(results):
    outs = [np.asarray(r["out"], np.float32) for r in results]
    return np.stack(
        [outs[0] + outs[1] + outs[2] + outs[3], outs[4] + outs[5] + outs[6] + outs[7]],
        axis=0,
    )


_NC_CACHE = {}


def kernel(x, Wq, Wk, Wv, Wo):
    in_maps = build_in_maps(x, Wq, Wk, Wv, Wo)
    if "nc" not in _NC_CACHE:
        _NC_CACHE["nc"] = build_nc()
    res = run_bass_kernel_spmd(_NC_CACHE["nc"], in_maps, core_ids=list(range(NCORES)))
    return gather_output(res.results)



# revision 34
# speedup vs baseline: 1.5154x; 1.5154x over previous
"""Multi-head causal attention (B=2, S=2048, E=1024, H=16, D=64) on 8 TRN2 cores.

Sharding: core c -> batch b = c // 4, head group g = c % 4 (4 heads each).
Each core computes q/k/v projections + RoPE + causal attention + its rows of
the Wo projection for its (batch, head-group); the host sums the 4 row-parallel
Wo partials per batch (the unshard step of row-parallel output projection).

Device layout notes:
  - x is passed pre-transposed per batch: xT [E, S] so the PE can contract
    over E (partition dim) for the projections.
  - q/k are computed transposed (qT/kT [64, S]) with head-pair fused weights
    so one [128, 512] PSUM tile holds [q_x1; q_x2; k_x1; k_x2] rows, where
    x1/x2 are the RoPE even/odd pair halves (weight columns pre-permuted on
    host so rotate-half applies).
  - RoPE: the projection PSUM is evacuated to f16 rs_all [128, h, S]; the
    rotate-half partition swap is done by an SBUF->SBUF DMA (rs_sw), keeping
    the DVE ops full-width f16: qk = rs*cos + rs_sw*sin.
  - scores are computed transposed, sT [k, q] = kT.T @ qT; softmax runs over
    the partition dim via an appended ones-column in the AV matmul (Z row).
    No max-subtraction: scores ~ N(0,1), exp is safe in fp32.
  - causal mask: DVE multiply of the exp'd scores by a 0/1 f16 mask on the
    diagonal k-tiles only (sliced to the triangular extent), not PE matmuls.
  - AV computes attnT [d, q]; per-head 1/Z applied at the av2 -> at8
    evacuation (DVE mul with a Pool-broadcast zb); Wo projection contracts
    head-dim chunks of attnT against Wo rows (fp16), accumulating out [s, e]
    tiles in PSUM.
"""

import sys

if "/opt/trn_rl_repo" not in sys.path:
    sys.path.insert(0, "/opt/trn_rl_repo")

import numpy as np

import concourse.bass as bass
import concourse.tile as tile
from concourse import bacc, mybir
from concourse.bass_utils import run_bass_kernel_spmd

B, S, E, H, D = 2, 2048, 1024, 16, 64
HPC = 4  # heads per core
NCORES = 8
SB = 512  # q/s block width
NSB = S // SB  # 4
KT = 128  # k tile (partition chunk of the sequence)
NKT = S // KT  # 16
ECH = E // 128  # 8 contraction chunks for the projections

f32 = mybir.dt.float32
f16 = mybir.dt.float16
bf16 = mybir.dt.bfloat16

ROPE_BASE = 10000.0

MM_LABELS = []

# tuning knobs (see sweep_cfg.py): engine/schedule choices
CFG = {
    "masks": "dve",       # pool | dve
    "evac_pre": "act",    # act | dve   (qk sb0/1 + v kc0-7 evacuations)
    "evac_tail": "dve",   # act | dve   (direct emit_wo_stl evacuations)
    "sched": "v2",        # fill | coarse | v2
    "ktorder": False,     # reorder k-tiles (prev, diag, rest)
    "cadence": 2,         # filler every N steps
    "loads": "v2",        # split | v2  (DMA ring/chunking strategy)
    "ropegran": 1,        # rope granularity in s-blocks (1 | 2)
    "qk2": False,         # interleaved head-pair chains for sb0
    "attnorder": "0132",  # attention block order
    "xtmerge": True,      # xT as one [128, ECH, S] tile vs 8 separate tiles
    "wqkfirst": True,     # issue the wqk/wv DMAs before the xT stream
    "wvlate": False,      # wv DMA after the sb0 xT chunks (first MM earlier)
    "wotail": False,      # feed wo(1)/wo(2) chunks into attn(3) as filler
    "qblock": 512,        # attention q-block width (512 | 256)
}


def build_nc(unroll=1):
    MM_LABELS.clear()
    nc = bacc.Bacc(
        "TRN2", target_bir_lowering=False, debug=False, enable_asserts=False
    )

    xT_d = nc.dram_tensor("xT", [E, S], f16, kind="ExternalInput")
    wqk_d = nc.dram_tensor("wqk", [E, HPC, 128], f16, kind="ExternalInput")
    wv_d = nc.dram_tensor("wv", [E, HPC * D], f16, kind="ExternalInput")
    wo_d = nc.dram_tensor("wo", [HPC * D, E], f16, kind="ExternalInput")
    cos_d = nc.dram_tensor("cos2", [128, S], f16, kind="ExternalInput")
    sin_d = nc.dram_tensor("sin2", [128, S], f16, kind="ExternalInput")
    mask_d = nc.dram_tensor("maskm", [128, 4, SB], f16, kind="ExternalInput")
    maskq_d = nc.dram_tensor("maskq", [128, 2, 256], f16, kind="ExternalInput")
    out_d = nc.dram_tensor("out", [S, E], f16, kind="ExternalOutput")

    with tile.TileContext(nc) as tc:
        with (
            tc.tile_pool(name="const", bufs=1) as constp,
            tc.tile_pool(name="qk", bufs=1) as qkp,
            tc.tile_pool(name="vb", bufs=1) as vbp,
            tc.tile_pool(name="at", bufs=1) as atp,
            tc.tile_pool(name="st", bufs=12) as stp,
            tc.tile_pool(name="tmp", bufs=4) as tmpp,
            tc.tile_pool(name="mm", bufs=2, space="PSUM") as mmp,
            tc.tile_pool(name="wps", bufs=2, space="PSUM") as wpsp,
            tc.tile_pool(name="acc", bufs=1, space="PSUM") as accp,
        ):
            # ---- constant tiles (DMAs issued per s-block, in consumption
            # order, so the first projection matmuls start within a few us) --
            xT_ap = xT_d.ap().rearrange("(eo p) s -> eo p s", p=128)
            if CFG["xtmerge"]:
                xT_all = constp.tile([128, ECH, S], f16, tag="xT", name="xT")
                xT = [
                    xT_all.rearrange("p e s -> e p s")[e] for e in range(ECH)
                ]
            else:
                xT_all = None
                xT = [
                    constp.tile([128, S], f16, tag=f"xT{e}", name=f"xT{e}")
                    for e in range(ECH)
                ]
            xT3_ap = xT_d.ap().rearrange("(eo p) s -> p eo s", p=128)
            wqk = constp.tile([128, ECH, HPC, 128], f16, tag="wqk")
            wqk_ap = wqk_d.ap().rearrange("(eo p) h m -> eo p h m", p=128)
            wv = constp.tile([128, ECH, HPC * D], f16, tag="wv")
            wv_ap = wv_d.ap().rearrange("(eo p) m -> eo p m", p=128)
            cos2 = constp.tile([128, S], f16, tag="cos2")
            sin2 = constp.tile([128, S], f16, tag="sin2")
            maskm = constp.tile([128, 4, SB], f16, tag="maskm")
            maskq = constp.tile([128, 2, 256], f16, tag="maskq")
            wo = constp.tile([128, 2, E], f16, tag="wo")

            def emit_loads(sb):
                # sync HWDGE ring carries the projection-critical stream
                # (wqk + xT, interleaved per e for sb0 so the first chains
                # start early); everything else rides the scalar HWDGE ring.
                cs = slice(sb * SB, (sb + 1) * SB)
                if CFG["loads"] == "v2":
                    if sb == 0 and CFG["wqkfirst"]:
                        nc.sync.dma_start(
                            out=wqk,
                            in_=wqk_d.ap().rearrange(
                                "(eo p) h m -> p eo h m", p=128
                            ),
                        )
                        if not CFG["wvlate"]:
                            nc.sync.dma_start(
                                out=wv,
                                in_=wv_d.ap().rearrange(
                                    "(eo p) m -> p eo m", p=128
                                ),
                            )
                    for e in range(ECH):
                        nc.sync.dma_start(out=xT[e][:, cs], in_=xT_ap[e][:, cs])
                    if sb == 0 and CFG["wqkfirst"] and CFG["wvlate"]:
                        nc.sync.dma_start(
                            out=wv,
                            in_=wv_d.ap().rearrange("(eo p) m -> p eo m", p=128),
                        )
                    nc.sync.dma_start(out=cos2[:, cs], in_=cos_d.ap()[:, cs])
                    nc.sync.dma_start(out=sin2[:, cs], in_=sin_d.ap()[:, cs])
                    if sb == 0:
                        nc.sync.dma_start(out=maskm, in_=mask_d.ap())
                        if not CFG["wqkfirst"]:
                            nc.sync.dma_start(
                                out=wqk,
                                in_=wqk_d.ap().rearrange(
                                    "(eo p) h m -> p eo h m", p=128
                                ),
                            )
                            nc.sync.dma_start(
                                out=wv,
                                in_=wv_d.ap().rearrange(
                                    "(eo p) m -> p eo m", p=128
                                ),
                            )
                    if sb == 1:
                        nc.sync.dma_start(
                            out=wo,
                            in_=wo_d.ap().rearrange("(c p) e -> p c e", p=128),
                        )
                    return
                if sb == 0:
                    nc.sync.dma_start(out=wqk[:, 0], in_=wqk_ap[0])
                    nc.sync.dma_start(out=xT[0][:, cs], in_=xT_ap[0][:, cs])
                    for e in range(1, ECH):
                        nc.sync.dma_start(out=wqk[:, e], in_=wqk_ap[e])
                        nc.sync.dma_start(out=xT[e][:, cs], in_=xT_ap[e][:, cs])
                    for e in range(ECH):
                        nc.scalar.dma_start(out=wv[:, e], in_=wv_ap[e])
                elif CFG["xtmerge"]:
                    nc.sync.dma_start(
                        out=xT_all[:, :, cs], in_=xT3_ap[:, :, cs]
                    )
                else:
                    for e in range(ECH):
                        nc.sync.dma_start(out=xT[e][:, cs], in_=xT_ap[e][:, cs])
                nc.scalar.dma_start(out=cos2[:, cs], in_=cos_d.ap()[:, cs])
                nc.scalar.dma_start(out=sin2[:, cs], in_=sin_d.ap()[:, cs])
                if sb == 0:
                    nc.scalar.dma_start(out=maskm, in_=mask_d.ap())
                if sb == 1:
                    nc.scalar.dma_start(
                        out=wo, in_=wo_d.ap().rearrange("(c p) e -> p c e", p=128)
                    )

            # rs_all rows (per head h): [q_x1(32); q_x2(32); k_x1(32); k_x2(32)]
            # rs_sw = partition-swapped copy (x1 <-> x2 blocks) via SBUF->SBUF
            # DMA so the rope DVE ops run full-width on 128 partitions.
            rs_all = qkp.tile([128, HPC, S], f16, tag="rs_all")
            rs_sw = qkp.tile([128, HPC, S], f16, tag="rs_sw")
            # qq[p] rows: qT of head 2p on partitions 0-63, head 2p+1 on 64-127
            # (kk[p] likewise) so each head's scores matmul operands share a
            # partition base. K=64 pairs land in disjoint PE row groups.
            qq = [
                qkp.tile([128, S], f16, tag=f"qq{p}", name=f"qq{p}")
                for p in range(2)
            ]
            kk = [
                qkp.tile([128, S], f16, tag=f"kk{p}", name=f"kk{p}")
                for p in range(2)
            ]

            def emit_qk_h(sb, h):
                cs = slice(sb * SB, (sb + 1) * SB)
                ps = mmp.tile([128, SB], f32, tag="mm", name="ps")
                for e in range(ECH):
                    MM_LABELS.append(f"qk{sb}h{h}e{e}")
                    nc.tensor.matmul(
                        out=ps,
                        lhsT=wqk[:, e, h, :],
                        rhs=xT[e][:, cs],
                        start=(e == 0),
                        stop=(e == ECH - 1),
                    )
                if CFG["evac_pre"] == "act":
                    nc.scalar.copy(out=rs_all[:, h, cs], in_=ps)
                else:
                    nc.vector.tensor_copy(out=rs_all[:, h, cs], in_=ps)

            def emit_qk_h2(sb, h0):
                # two head-chains interleaved over e so each arriving xT chunk
                # feeds ~430ns of PE work (matches the cold-start DMA rate)
                cs = slice(sb * SB, (sb + 1) * SB)
                ps0 = mmp.tile([128, SB], f32, tag="mm", name="ps")
                ps1 = mmp.tile([128, SB], f32, tag="mm", name="ps")
                for e in range(ECH):
                    for ps, h in ((ps0, h0), (ps1, h0 + 1)):
                        MM_LABELS.append(f"qk2_{sb}h{h}e{e}")
                        nc.tensor.matmul(
                            out=ps,
                            lhsT=wqk[:, e, h, :],
                            rhs=xT[e][:, cs],
                            start=(e == 0),
                            stop=(e == ECH - 1),
                        )
                if CFG["evac_pre"] == "act":
                    nc.scalar.copy(out=rs_all[:, h0, cs], in_=ps0)
                    nc.scalar.copy(out=rs_all[:, h0 + 1, cs], in_=ps1)
                else:
                    nc.vector.tensor_copy(out=rs_all[:, h0, cs], in_=ps0)
                    nc.vector.tensor_copy(out=rs_all[:, h0 + 1, cs], in_=ps1)

            def emit_rope(sb, eng=None, width=1):
                # eng=nc.gpsimd offloads the rope math to the (idle) Pool
                # engine; the DVE is the contended resource mid-kernel.
                eng = eng or nc.vector
                hs = slice(sb * SB, (sb + width) * SB)
                # partition-block swap x1<->x2 for q and k halves
                for dst, src in ((0, 32), (32, 0), (64, 96), (96, 64)):
                    nc.sync.dma_start(
                        out=rs_sw[dst : dst + 32, :, hs],
                        in_=rs_all[src : src + 32, :, hs],
                    )
                for h in range(HPC):
                    p, hhalf = h // 2, (h % 2) * 64
                    t1 = tmpp.tile([128, width * SB], f16, tag="t1", name="t1")
                    t2 = tmpp.tile([128, width * SB], f16, tag="t2", name="t2")
                    eng.tensor_mul(t1, rs_all[:, h, hs], cos2[:, hs])
                    eng.tensor_mul(t2, rs_sw[:, h, hs], sin2[:, hs])
                    eng.tensor_add(
                        qq[p][hhalf : hhalf + 64, hs], t1[0:64, :], t2[0:64, :]
                    )
                    eng.tensor_add(
                        kk[p][hhalf : hhalf + 64, hs], t1[64:128, :], t2[64:128, :]
                    )

            # v_big free layout per k-chunk: 4 heads x [v_h (64) | one (1)]
            v_big = vbp.tile([128, NKT, HPC * 65], f16, tag="vbig")
            ones_cols = v_big.rearrange("p n (h m) -> p n h m", h=HPC)[
                :, :, :, 64:65
            ]
            nc.vector.memset(ones_cols, 1.0)

            def emit_v_kc(kc):
                vps = mmp.tile([128, HPC * D], f32, tag="mm", name="vps")
                for e in range(ECH):
                    MM_LABELS.append(f"v{kc}e{e}")
                    nc.tensor.matmul(
                        out=vps,
                        lhsT=xT[e][:, kc * KT : (kc + 1) * KT],
                        rhs=wv[:, e, :],
                        start=(e == 0),
                        stop=(e == ECH - 1),
                    )
                veng = nc.scalar if CFG["evac_pre"] == "act" else nc.vector
                vcopy = veng.copy if CFG["evac_pre"] == "act" else veng.tensor_copy
                vcopy(
                    out=v_big.rearrange("p n (h m) -> p n h m", h=HPC)[
                        :, kc, :, 0:64
                    ],
                    in_=vps.rearrange("p (h m) -> p h m", h=HPC),
                )

            # ---- phase C: attention per (q block, head pair) --------------------
            # attnT tiles: at8[c][qb] rows = hd chunk c (2 heads x 64), cols = q
            at8 = {}
            for c in range(2):
                for qb in range(NSB):
                    at8[(c, qb)] = atp.tile(
                        [128, SB], f16, tag=f"at{c}_{qb}", name=f"at{c}_{qb}"
                    )

            def emit_attn(qb, filler=None):
                qs = slice(qb * SB, (qb + 1) * SB)
                n_k = 4 * (qb + 1)
                # k-tile order: previous block first (its rope/v are oldest),
                # then the diagonal (masked) tiles -- their st has an extra DVE
                # mask hop, so keep their AV matmuls away from the tail -- and
                # the rest last.
                if qb == 0 or not CFG["ktorder"]:
                    kt_order = list(range(n_k))
                else:
                    prev = list(range(4 * qb - 4, 4 * qb))
                    diag = list(range(4 * qb, n_k))
                    rest = list(range(4 * qb - 4))
                    kt_order = prev + diag + rest
                for p in range(2):
                    # one wide [128, 1024] PSUM pair-tile per head pair: both
                    # heads' scores live side by side so a single ACT exp
                    # covers them (halves exp instructions and sem hops)
                    av2 = accp.tile([128, 2 * SB], f32, tag="acc", name="av2")
                    # Software pipeline: emit the AV matmul for chunk kt only
                    # LAG steps after its scores matmul, so the PE (strict
                    # in-order queue) never head-of-line blocks on the ACT exp.
                    LAG = 2
                    sts_buf = {}
                    for step in range(n_k + LAG):
                        # keep the PE packed: the exp pacing leaves ~190ns of
                        # PE slack per step; drip one queued ~0.6us chunk
                        # every other step
                        if (
                            filler is not None
                            and step % CFG["cadence"] == CFG["cadence"] - 1
                        ):
                            filler()
                        if step < n_k:
                            kt = kt_order[step]
                            j = kt - 4 * qb
                            kts = slice(kt * KT, (kt + 1) * KT)
                            ps2 = wpsp.tile(
                                [128, 2 * SB], f32, tag="wps", name="ps2"
                            )
                            for i in range(2):
                                half = i * 64
                                MM_LABELS.append(f"sc_q{qb}p{p}kt{kt}i{i}")
                                nc.tensor.matmul(
                                    out=ps2[:, i * SB : (i + 1) * SB],
                                    lhsT=kk[p][half : half + 64, kts],
                                    rhs=qq[p][half : half + 64, qs],
                                    start=True,
                                    stop=True,
                                )
                            st_t = stp.tile(
                                [128, 2 * SB], f16, tag="st", name="st_t"
                            )
                            nc.scalar.activation(
                                out=st_t,
                                in_=ps2,
                                func=mybir.ActivationFunctionType.Exp,
                                scale=0.125,
                            )
                            if j >= 0:
                                # causal mask: zero the exp'd scores where
                                # k > q; only columns < 128*(j+1) can be
                                # masked, so slice to the triangular extent.
                                mext = KT * (j + 1)
                                meng = (
                                    nc.gpsimd
                                    if CFG["masks"] == "pool"
                                    else nc.vector
                                )
                                for i in range(2):
                                    meng.tensor_mul(
                                        st_t[:, i * SB : i * SB + mext],
                                        st_t[:, i * SB : i * SB + mext],
                                        maskm[:, j, 0:mext],
                                    )
                            sts_buf[kt] = st_t
                        if step >= LAG:
                            kt = kt_order[step - LAG]
                            st_t = sts_buf.pop(kt)
                            for i in range(2):
                                h = 2 * p + i
                                MM_LABELS.append(f"av_q{qb}p{p}kt{kt}i{i}")
                                nc.tensor.matmul(
                                    out=av2[0:65, i * SB : (i + 1) * SB],
                                    lhsT=v_big[:, kt, h * 65 : (h + 1) * 65],
                                    rhs=st_t[:, i * SB : (i + 1) * SB],
                                    start=(step - LAG == 0),
                                    stop=(step - LAG == n_k - 1),
                                )
                    # normalize: attnT = av[0:64] / Z  (Z = av row 64), with
                    # 1/Z broadcast to a [64, SB]-pair tile zb on Pool.
                    zrec = tmpp.tile([1, 2 * SB], f32, tag="zr", name="zrec")
                    nc.vector.reciprocal(out=zrec, in_=av2[64:65, :])
                    zb0 = tmpp.tile([64, SB], f32, tag="zb0", name="zb0")
                    zb1 = tmpp.tile([64, SB], f32, tag="zb1", name="zb1")
                    nc.gpsimd.partition_broadcast(zb0, zrec[:, 0:SB])
                    nc.gpsimd.partition_broadcast(zb1, zrec[:, SB : 2 * SB])
                    nc.vector.tensor_mul(
                        at8[(p, qb)][0:64, :], av2[0:64, 0:SB], zb0
                    )
                    nc.vector.tensor_mul(
                        at8[(p, qb)][64:128, :], av2[0:64, SB : 2 * SB], zb1
                    )

            # ---- q256 attention variant ----------------------------------------
            at9 = {}
            if CFG["qblock"] == 256:
                for c in range(2):
                    for qb2 in range(8):
                        at9[(c, qb2)] = atp.tile(
                            [128, 256], f16, tag=f"at9_{c}_{qb2}",
                            name=f"at9_{c}_{qb2}",
                        )

            def emit_attn2(qb2):
                SQ = 256
                qs = slice(qb2 * SQ, (qb2 + 1) * SQ)
                n_k = 2 * (qb2 + 1)  # 128-wide k tiles
                n_g = qb2 + 1       # groups of 2 k-tiles
                for p in range(2):
                    av4 = accp.tile([128, 2 * SQ], f32, tag="acc", name="av4")
                    LAG = 2
                    sts_buf = {}
                    for step in range(n_g + LAG):
                        if step < n_g:
                            g = step
                            ps2 = wpsp.tile(
                                [128, 4 * SQ], f32, tag="wps", name="ps2"
                            )
                            for u in range(2):  # kt within group
                                kt = 2 * g + u
                                kts = slice(kt * KT, (kt + 1) * KT)
                                for i in range(2):
                                    half = i * 64
                                    MM_LABELS.append(
                                        f"sc2_q{qb2}p{p}kt{kt}i{i}"
                                    )
                                    nc.tensor.matmul(
                                        out=ps2[
                                            :,
                                            (2 * u + i) * SQ : (2 * u + i + 1)
                                            * SQ,
                                        ],
                                        lhsT=kk[p][half : half + 64, kts],
                                        rhs=qq[p][half : half + 64, qs],
                                        start=True,
                                        stop=True,
                                    )
                            st_t = stp.tile(
                                [128, 4 * SQ], f16, tag="st", name="st_t"
                            )
                            nc.scalar.activation(
                                out=st_t,
                                in_=ps2,
                                func=mybir.ActivationFunctionType.Exp,
                                scale=0.125,
                            )
                            if g == n_g - 1:
                                # diagonal group: kt=2*qb2 keep iff c>=t
                                # (only cols < 128 affected), kt=2*qb2+1
                                # keep iff c>=t+128 (all cols affected)
                                for i in range(2):
                                    nc.vector.tensor_mul(
                                        st_t[:, i * SQ : i * SQ + KT],
                                        st_t[:, i * SQ : i * SQ + KT],
                                        maskq[:, 0, 0:KT],
                                    )
                                    nc.vector.tensor_mul(
                                        st_t[:, (2 + i) * SQ : (3 + i) * SQ],
                                        st_t[:, (2 + i) * SQ : (3 + i) * SQ],
                                        maskq[:, 1, :],
                                    )
                            sts_buf[g] = st_t
                        if step >= LAG:
                            g = step - LAG
                            st_t = sts_buf.pop(g)
                            for u in range(2):
                                kt = 2 * g + u
                                for i in range(2):
                                    h = 2 * p + i
                                    MM_LABELS.append(
                                        f"av2_q{qb2}p{p}kt{kt}i{i}"
                                    )
                                    nc.tensor.matmul(
                                        out=av4[0:65, i * SQ : (i + 1) * SQ],
                                        lhsT=v_big[
                                            :, kt, h * 65 : (h + 1) * 65
                                        ],
                                        rhs=st_t[
                                            :,
                                            (2 * u + i) * SQ : (2 * u + i + 1)
                                            * SQ,
                                        ],
                                        start=(kt == 0),
                                        stop=(kt == n_k - 1),
                                    )
                    zrec = tmpp.tile([1, 2 * SQ], f32, tag="zr", name="zrec")
                    nc.vector.reciprocal(out=zrec, in_=av4[64:65, :])
                    zb0 = tmpp.tile([64, SQ], f32, tag="zb0", name="zb0")
                    zb1 = tmpp.tile([64, SQ], f32, tag="zb1", name="zb1")
                    nc.gpsimd.partition_broadcast(zb0, zrec[:, 0:SQ])
                    nc.gpsimd.partition_broadcast(zb1, zrec[:, SQ : 2 * SQ])
                    nc.vector.tensor_mul(
                        at9[(p, qb2)][0:64, :], av4[0:64, 0:SQ], zb0
                    )
                    nc.vector.tensor_mul(
                        at9[(p, qb2)][64:128, :], av4[0:64, SQ : 2 * SQ], zb1
                    )

            def emit_wo2(qb2):
                for stl in range(2):
                    rows = qb2 * 256 + stl * KT
                    for eb in range(2):
                        pw = mmp.tile([128, SB], f32, tag="mm", name="pw")
                        for c in range(2):
                            MM_LABELS.append(f"wo2_{qb2}s{stl}eb{eb}c{c}")
                            nc.tensor.matmul(
                                out=pw,
                                lhsT=at9[(c, qb2)][
                                    :, stl * KT : (stl + 1) * KT
                                ],
                                rhs=wo[:, c, eb * SB : (eb + 1) * SB],
                                start=(c == 0),
                                stop=(c == 1),
                            )
                        ot = stp.tile(
                            [128, SB], f16, tag="ot", name="ot", bufs=3
                        )
                        nc.vector.tensor_copy(out=ot, in_=pw)
                        nc.sync.dma_start(
                            out=out_d.ap()[
                                rows : rows + KT, eb * SB : (eb + 1) * SB
                            ],
                            in_=ot,
                        )

            # ---- phase D: output projection (row-parallel partial) -------------
            def emit_wo_stl(qb, stl):
                rows = qb * SB + stl * KT
                for eb in range(2):
                    pw = mmp.tile([128, SB], f32, tag="mm", name="pw")
                    for c in range(2):
                        MM_LABELS.append(f"wo{qb}s{stl}eb{eb}c{c}")
                        nc.tensor.matmul(
                            out=pw,
                            lhsT=at8[(c, qb)][:, stl * KT : (stl + 1) * KT],
                            rhs=wo[:, c, eb * SB : (eb + 1) * SB],
                            start=(c == 0),
                            stop=(c == 1),
                        )
                    ot = stp.tile([128, SB], f16, tag="ot", name="ot", bufs=3)
                    if CFG["evac_tail"] == "act":
                        nc.scalar.copy(out=ot, in_=pw)
                    else:
                        nc.vector.tensor_copy(out=ot, in_=pw)
                    nc.sync.dma_start(
                        out=out_d.ap()[rows : rows + KT, eb * SB : (eb + 1) * SB],
                        in_=ot,
                    )

            # ---- emission schedule ---------------------------------------------
            # The attention phase is ACT(exp)-paced (~1.04us/step vs ~0.85us of
            # PE work per step), so queued proj/Wo matmul work is drip-fed
            # into the attention step loop as small (~0.5us) PE filler chunks.
            # attn order 0,1,3,2 keeps the tail (last attention block + its
            # Wo) short. unroll > 1 repeats the kernel for overhead-free
            # timing.
            from collections import deque

            fill = deque()

            def filler():
                if fill:
                    fill.popleft()()

            def drain():
                while fill:
                    fill.popleft()()

            def qk_fill_chunks(sb, h):
                cs = slice(sb * SB, (sb + 1) * SB)
                cell = {}

                def chunk(e0, e1):
                    def go():
                        if e0 == 0:
                            cell["ps"] = mmp.tile(
                                [128, SB], f32, tag="mm", name="ps"
                            )
                        for e in range(e0, e1):
                            MM_LABELS.append(f"qkF{sb}h{h}e{e}")
                            nc.tensor.matmul(
                                out=cell["ps"],
                                lhsT=wqk[:, e, h, :],
                                rhs=xT[e][:, cs],
                                start=(e == 0),
                                stop=(e == ECH - 1),
                            )
                        if e1 == ECH:
                            nc.vector.tensor_copy(
                                out=rs_all[:, h, cs], in_=cell["ps"]
                            )

                    return go

                return [chunk(0, 3), chunk(3, 6), chunk(6, 8)]

            def v_fill_chunks(kc):
                cell = {}

                def chunk(e0, e1):
                    def go():
                        if e0 == 0:
                            cell["ps"] = mmp.tile(
                                [128, HPC * D], f32, tag="mm", name="vps"
                            )
                        for e in range(e0, e1):
                            MM_LABELS.append(f"vF{kc}e{e}")
                            nc.tensor.matmul(
                                out=cell["ps"],
                                lhsT=xT[e][:, kc * KT : (kc + 1) * KT],
                                rhs=wv[:, e, :],
                                start=(e == 0),
                                stop=(e == ECH - 1),
                            )
                        if e1 == ECH:
                            nc.vector.tensor_copy(
                                out=v_big.rearrange(
                                    "p n (h m) -> p n h m", h=HPC
                                )[:, kc, :, 0:64],
                                in_=cell["ps"].rearrange(
                                    "p (h m) -> p h m", h=HPC
                                ),
                            )

                    return go

                return [chunk(0, 4), chunk(4, 8)]

            def wo_fill_chunks(qb, stl):
                def chunk(eb):
                    def go():
                        pw = mmp.tile([128, SB], f32, tag="mm", name="pw")
                        for c in range(2):
                            MM_LABELS.append(f"woF{qb}s{stl}eb{eb}c{c}")
                            nc.tensor.matmul(
                                out=pw,
                                lhsT=at8[(c, qb)][:, stl * KT : (stl + 1) * KT],
                                rhs=wo[:, c, eb * SB : (eb + 1) * SB],
                                start=(c == 0),
                                stop=(c == 1),
                            )
                        ot = stp.tile(
                            [128, SB], f16, tag="ot", name="ot", bufs=3
                        )
                        nc.vector.tensor_copy(out=ot, in_=pw)
                        rows = qb * SB + stl * KT
                        nc.sync.dma_start(
                            out=out_d.ap()[
                                rows : rows + KT, eb * SB : (eb + 1) * SB
                            ],
                            in_=ot,
                        )

                    return go

                return [chunk(0), chunk(1)]

            for _ in range(unroll):
                if CFG["qblock"] == 256:
                    emit_loads(0)
                    for h in range(HPC):
                        emit_qk_h(0, h)
                    for kc in range(4):
                        emit_v_kc(kc)
                    emit_loads(1)
                    for h in range(HPC):
                        emit_qk_h(1, h)
                    for kc in range(4, 8):
                        emit_v_kc(kc)
                    emit_rope(0, width=2)
                    emit_loads(2)
                    for h in range(HPC):
                        emit_qk_h(2, h)
                    for kc in range(8, 12):
                        emit_v_kc(kc)
                    emit_attn2(0)
                    emit_attn2(1)
                    emit_loads(3)
                    for h in range(HPC):
                        emit_qk_h(3, h)
                    for kc in range(12, 16):
                        emit_v_kc(kc)
                    emit_attn2(2)
                    emit_attn2(3)
                    emit_rope(2, width=2)
                    emit_wo2(0)
                    emit_attn2(4)
                    emit_wo2(1)
                    emit_attn2(5)
                    emit_wo2(2)
                    emit_attn2(6)
                    emit_wo2(3)
                    emit_attn2(7)
                    emit_wo2(4)
                    emit_wo2(5)
                    emit_wo2(6)
                    emit_wo2(7)
                    continue
                if CFG["sched"] == "fill":
                    emit_loads(0)
                    emit_loads(1)
                    emit_qk_h2(0, 0)
                    emit_qk_h2(0, 2)
                    emit_rope(0)
                    for kc in range(4):
                        emit_v_kc(kc)
                    for h in range(HPC):
                        emit_qk_h(1, h)
                    emit_rope(1)
                    for kc in range(4, 8):
                        emit_v_kc(kc)
                    emit_loads(2)
                    emit_loads(3)
                    for h in range(HPC):
                        fill.extend(qk_fill_chunks(2, h))
                    fill.append(lambda: emit_rope(2))
                    for kc in range(8, 12):
                        fill.extend(v_fill_chunks(kc))
                    emit_attn(0, filler)
                    for h in range(HPC):
                        fill.extend(qk_fill_chunks(3, h))
                    fill.append(lambda: emit_rope(3))
                    for kc in range(12, 16):
                        fill.extend(v_fill_chunks(kc))
                    emit_attn(1, filler)
                    for s in range(4):
                        fill.extend(wo_fill_chunks(0, s))
                    emit_attn(3, filler)
                    for s in range(4):
                        fill.extend(wo_fill_chunks(1, s))
                        fill.extend(wo_fill_chunks(3, s))
                    emit_attn(2, filler)
                    drain()
                    for s in range(4):
                        emit_wo_stl(2, s)
                elif CFG["sched"] == "v2":
                    emit_loads(0)
                    for h in range(HPC):
                        emit_qk_h(0, h)
                    for kc in range(4):
                        emit_v_kc(kc)
                    emit_loads(1)
                    for h in range(HPC):
                        emit_qk_h(1, h)
                    for kc in range(4, 8):
                        emit_v_kc(kc)
                    emit_rope(0, width=2)
                    emit_loads(2)
                    for h in range(HPC):
                        emit_qk_h(2, h)
                    for kc in range(8, 12):
                        emit_v_kc(kc)
                    emit_attn(0)
                    emit_loads(3)
                    for h in range(HPC):
                        emit_qk_h(3, h)
                    for kc in range(12, 16):
                        emit_v_kc(kc)
                    emit_rope(2, width=2)
                    emit_attn(1)
                    for s in range(4):
                        emit_wo_stl(0, s)
                    emit_attn(2)
                    if CFG["wotail"]:
                        for s in range(4):
                            fill.extend(wo_fill_chunks(1, s))
                            fill.extend(wo_fill_chunks(2, s))
                        emit_attn(3, filler)
                        drain()
                    else:
                        for s in range(4):
                            emit_wo_stl(1, s)
                        emit_attn(3)
                        for s in range(4):
                            emit_wo_stl(2, s)
                    for s in range(4):
                        emit_wo_stl(3, s)
                else:  # coarse
                    emit_loads(0)
                    emit_loads(1)
                    emit_qk_h2(0, 0)
                    emit_qk_h2(0, 2)
                    emit_rope(0)
                    for kc in range(4):
                        emit_v_kc(kc)
                    for h in range(HPC):
                        emit_qk_h(1, h)
                    emit_rope(1)
                    for kc in range(4, 8):
                        emit_v_kc(kc)
                    emit_loads(2)
                    emit_loads(3)
                    for h in range(HPC):
                        emit_qk_h(2, h)
                    emit_rope(2)
                    for kc in range(8, 12):
                        emit_v_kc(kc)
                    emit_attn(0)
                    for h in range(HPC):
                        emit_qk_h(3, h)
                    emit_rope(3)
                    for kc in range(12, 16):
                        emit_v_kc(kc)
                    emit_attn(1)
                    for s in range(4):
                        emit_wo_stl(0, s)
                    emit_attn(3)
                    for s in range(4):
                        emit_wo_stl(1, s)
                    emit_attn(2)
                    for s in range(4):
                        emit_wo_stl(3, s)
                    for s in range(4):
                        emit_wo_stl(2, s)

    nc.compile()
    return nc


def build_in_maps(x, Wq, Wk, Wv, Wo):
    x = np.asarray(x, np.float32)
    Wq = np.asarray(Wq, np.float32)
    Wk = np.asarray(Wk, np.float32)
    Wv = np.asarray(Wv, np.float32)
    Wo = np.asarray(Wo, np.float32)

    # RoPE tables in rotate-half layout ([32] pair-frequencies, duplicated)
    inv = 1.0 / (ROPE_BASE ** (np.arange(0, D, 2, dtype=np.float64) / D))  # [32]
    ang = inv[:, None] * np.arange(S, dtype=np.float64)[None, :]  # [32, S]
    cos_t = np.cos(ang).astype(np.float32)
    sin_t = np.sin(ang).astype(np.float32)
    cos2 = np.concatenate([cos_t, cos_t, cos_t, cos_t], 0)  # [128, S]
    # rs_sw rows already hold the swapped operand; sign pattern applies to
    # the sin table: row block x1 gets -sin (o1 = x1 cos - x2 sin), x2 +sin.
    sin2 = np.concatenate([-sin_t, sin_t, -sin_t, sin_t], 0)  # [128, S]

    # 0/1 causal mask per diagonal chunk j: keep (c >= t + j*KT) where t is
    # the key row within the tile and c the query column within the block.
    tt = np.arange(128)[:, None]
    cc = np.arange(SB)[None, :]
    maskm = np.ascontiguousarray(
        np.stack([(cc >= tt + j * KT) for j in range(4)], axis=1)
    ).astype(np.float16)  # [128, 4, SB]

    # weight column permutation: even pair-elements then odd (rotate-half)
    perm = np.concatenate([np.arange(0, D, 2), np.arange(1, D, 2)])

    in_maps = []
    for core in range(NCORES):
        b, g = core // HPC, core % HPC
        wqk = np.empty((E, HPC, 128), np.float32)
        for i in range(HPC):
            h = g * HPC + i
            wqk[:, i, 0:64] = Wq[:, h * D : (h + 1) * D][:, perm]
            wqk[:, i, 64:128] = Wk[:, h * D : (h + 1) * D][:, perm]
        in_maps.append(
            {
                "xT": np.ascontiguousarray(x[b].T).astype(np.float16),
                "wqk": wqk.astype(np.float16),
                "wv": np.ascontiguousarray(
                    Wv[:, g * HPC * D : (g + 1) * HPC * D]
                ).astype(np.float16),
                "wo": np.ascontiguousarray(
                    Wo[g * HPC * D : (g + 1) * HPC * D, :]
                ).astype(np.float16),
                "cos2": cos2.astype(np.float16),
                "sin2": sin2.astype(np.float16),
                "maskm": maskm,
            }
        )
    return in_maps


def gather_output(results):
    outs = [np.asarray(r["out"], np.float32) for r in results]
    return np.stack(
        [outs[0] + outs[1] + outs[2] + outs[3], outs[4] + outs[5] + outs[6] + outs[7]],
        axis=0,
    )


_NC_CACHE = {}


def kernel(x, Wq, Wk, Wv, Wo):
    in_maps = build_in_maps(x, Wq, Wk, Wv, Wo)
    if "nc" not in _NC_CACHE:
        _NC_CACHE["nc"] = build_nc()
    res = run_bass_kernel_spmd(_NC_CACHE["nc"], in_maps, core_ids=list(range(NCORES)))
    return gather_output(res.results)
